# revision 3
# baseline (speedup 1.0000x reference)
"""Trainium2 Bass kernel for the 2-layer hetero-GCN + linear edge decoder.

Math restructuring (exact, up to fp reassociation):
  hetero_conv: out = sum_r nd_r*(A_r @ (ns_r*x)) @ W_r + sum_r b_r
    -> per-edge weight w_e = ns_r[src]*nd_r[dst] folded into a one-hot
       scatter matrix S; aggregation z_r = S^T @ gathered(x) runs on the
       TensorEngine; the W_r matmul happens once per dst-tile.
  decoder has NO nonlinearity between Wp1 and Wp2, so
    score[e] = u[src_e] + v[dst_e] + c,  u = feat @ (Wp1@Wp2)[:256],
    v = feat @ (Wp1@Wp2)[256:], folded into the layer-B weight matmul.

Sharding: dst-nodes partitioned into 128-node tiles, 49 tiles/core x 8
cores; x/h tables replicated; AllGather of the layer-A activations and of
the tiny (u,v) table; decoder edges sharded evenly.
"""
import os
import sys

for _p in ("/opt/trn_rl_repo", "/root/.axon_site/_ro/trn_rl_repo"):
    if os.path.isdir(_p) and _p not in sys.path:
        sys.path.append(_p)

import numpy as np

import concourse.bass as bass
import concourse.bacc as bacc
import concourse.mybir as mybir
import concourse.tile as tile
from concourse.bass_utils import run_bass_kernel_spmd
from concourse.masks import make_identity

P = 128
NC = 8
F32 = mybir.dt.float32
DEBUG = False
I16 = mybir.dt.int16


def _deg_norm(idx, n):
    deg = np.bincount(idx, minlength=n).astype(np.float32)
    out = np.zeros(n, np.float32)
    nz = deg > 0
    out[nz] = 1.0 / np.sqrt(np.maximum(deg[nz], 1.0))
    return out


def _wrap16(a):
    # [G, S] int16 -> [16, G*S/16] wrapped (idx i of each group -> [i%16, i//16])
    G, S = a.shape
    return a.reshape(G, S // 16, 16).transpose(2, 0, 1).reshape(16, G * (S // 16))


def _pack_edges(src_f, rel_f, dst_f, w_f, split, tpc, sec, ntiles):
    """Pack edges into per-core gather-idx + meta arrays.

    Returns idxA, idxB [NC][128, tpc*8*sec/16] int16 (wrapped+replicated),
    meta [NC][128, tpc*8*2*pch] f32 (dst_local chunks | w chunks).
    """
    pch2 = 2 * sec // P
    tile_id = dst_f >> 7
    half = (src_f >= split).astype(np.int64)
    order = np.lexsort((half, rel_f, tile_id))
    so, ro, do_, wo, ho = (a[order] for a in (src_f, rel_f, dst_f, w_f, half))
    to = tile_id[order]
    key = (to * 8 + ro) * 2 + ho
    ngroup = ntiles * 8 * 2
    counts = np.bincount(key, minlength=ngroup)
    assert counts.max() <= sec, (counts.max(), sec)
    starts = np.zeros(ngroup, np.int64)
    np.cumsum(counts[:-1], out=starts[1:])
    rank = np.arange(len(so)) - starts[key]
    # slot within (tile, rel) group of 2*sec
    gslot = ho * sec + rank
    gidx = to * 8 + ro  # global group id (tile-major)
    idx_val = (so - ho * split).astype(np.int16)

    idxA = np.zeros((ntiles * 8, sec), np.int16)
    idxB = np.zeros((ntiles * 8, sec), np.int16)
    mA = ho == 0
    idxA[gidx[mA], rank[mA]] = idx_val[mA]
    idxB[gidx[~mA], rank[~mA]] = idx_val[~mA]
    dst_local = np.zeros((ntiles * 8, 2 * sec), np.float32)
    w_slot = np.zeros((ntiles * 8, 2 * sec), np.float32)
    dst_local[gidx, gslot] = (do_ & 127).astype(np.float32)
    w_slot[gidx, gslot] = wo

    gpc = tpc * 8  # groups per core
    idxA_c, idxB_c, meta_c = [], [], []
    for c in range(NC):
        sl = slice(c * gpc, (c + 1) * gpc)
        ia = np.tile(_wrap16(idxA[sl]), (8, 1))
        ib = np.tile(_wrap16(idxB[sl]), (8, 1))
        # meta: per group block of 2*pch2 cols: [dst chunks pch2 | w chunks pch2]
        d = dst_local[sl].reshape(gpc, pch2, P).transpose(2, 0, 1)  # [128, gpc, pch2]
        w_ = w_slot[sl].reshape(gpc, pch2, P).transpose(2, 0, 1)
        meta = np.concatenate([d, w_], axis=2).reshape(P, gpc * 2 * pch2)
        idxA_c.append(np.ascontiguousarray(ia))
        idxB_c.append(np.ascontiguousarray(ib))
        meta_c.append(np.ascontiguousarray(meta))
    return idxA_c, idxB_c, meta_c


def _build(dims):
    (tpc, sec, xcols, nsl_a, ksizes_a, epc, nb) = (
        dims["tpc"], dims["sec"], dims["xcols"], dims["nsl_a"],
        dims["ksizes_a"], dims["epc"], dims["nb"])
    ntiles = tpc * NC
    npad = ntiles * P
    half_h = npad // 2
    split_x = dims["split_x"]
    pch2 = 2 * sec // P
    secw = sec // 16
    HC = 512
    R = 8
    nc = bacc.Bacc("TRN2", target_bir_lowering=False, debug=False)

    xA = nc.declare_dram_parameter("xA", [split_x, xcols], F32, isOutput=False)
    xB = nc.declare_dram_parameter("xB", [dims["n"] - split_x, xcols], F32, isOutput=False)
    WA = nc.declare_dram_parameter("WA", [R, nsl_a, P, 256], F32, isOutput=False)
    WB = nc.declare_dram_parameter("WB", [R, 4, P, 16], F32, isOutput=False)
    bias_rep = nc.declare_dram_parameter("bias_rep", [P, HC], F32, isOutput=False)
    uvb_rep = nc.declare_dram_parameter("uvb_rep", [P, 16], F32, isOutput=False)
    iota_rep = nc.declare_dram_parameter("iota_rep", [P, P], F32, isOutput=False)
    idxa_a = nc.declare_dram_parameter("idxa_a", [P, tpc * 8 * secw], I16, isOutput=False)
    idxb_a = nc.declare_dram_parameter("idxb_a", [P, tpc * 8 * secw], I16, isOutput=False)
    meta_a = nc.declare_dram_parameter("meta_a", [P, tpc * 8 * 2 * pch2], F32, isOutput=False)
    idxa_b = nc.declare_dram_parameter("idxa_b", [P, tpc * 8 * secw], I16, isOutput=False)
    idxb_b = nc.declare_dram_parameter("idxb_b", [P, tpc * 8 * secw], I16, isOutput=False)
    meta_b = nc.declare_dram_parameter("meta_b", [P, tpc * 8 * 2 * pch2], F32, isOutput=False)
    idx_u = nc.declare_dram_parameter("idx_u", [P, nb * 64], I16, isOutput=False)
    idx_v = nc.declare_dram_parameter("idx_v", [P, nb * 64], I16, isOutput=False)
    msk_u = nc.declare_dram_parameter("msk_u", [P, nb * 64], F32, isOutput=False)
    msk_v = nc.declare_dram_parameter("msk_v", [P, nb * 64], F32, isOutput=False)
    score_out = nc.declare_dram_parameter("score_out", [nb * 1024, 8], F32, isOutput=True)
    if DEBUG:
        h_out = nc.declare_dram_parameter("h_out", [tpc * P, 512], F32, isOutput=True)
        uv_out = nc.declare_dram_parameter("uv_out", [tpc * P, 16], F32, isOutput=True)

    with tile.TileContext(nc) as tc:
        with (
            tc.tile_pool(name="cpool", bufs=1) as cp,
            tc.tile_pool(name="dram", bufs=1, space="DRAM") as dp,
        ):
            h_shard = dp.tile([tpc * P, HC], F32)
            h_full = dp.tile([npad, HC], F32, addr_space="Shared")
            uv_shard = dp.tile([tpc * P, 16], F32)
            uv_full = dp.tile([npad, 16], F32, addr_space="Shared")
            uv_pack = dp.tile([npad // 2, 64], F32)

            ident = cp.tile([P, P], F32)
            make_identity(nc, ident[:])
            iota_t = cp.tile([P, P], F32)
            nc.sync.dma_start(out=iota_t[:], in_=iota_rep[:, :])
            bias_t = cp.tile([P, HC], F32)
            nc.sync.dma_start(out=bias_t[:], in_=bias_rep[:, :])
            uvb_t = cp.tile([P, 16], F32)
            nc.sync.dma_start(out=uvb_t[:], in_=uvb_rep[:, :])

            # ---------------- layer A ----------------
            with (
                tc.tile_pool(name="wpool", bufs=1) as wp,
                tc.tile_pool(name="gpool", bufs=3) as gp,
                tc.tile_pool(name="spool", bufs=4) as sp,
                tc.tile_pool(name="zpool", bufs=2) as zp,
                tc.tile_pool(name="ztpool", bufs=2) as ztp_p,
                tc.tile_pool(name="mpool", bufs=2) as mp,
                tc.tile_pool(name="pspool", bufs=2, space="PSUM") as ps,
                tc.tile_pool(name="ps2pool", bufs=2, space="PSUM") as ps2,
                tc.tile_pool(name="ps3pool", bufs=1, space="PSUM") as ps3,
            ):
                wa_t = []
                for r in range(R):
                    for k in range(nsl_a):
                        w_rk = wp.tile([P, 256], F32, tag=f"wa{r}_{k}")
                        nc.sync.dma_start(out=w_rk[:], in_=WA[r, k, :, :])
                        wa_t.append(w_rk)
                for t in range(tpc):
                    ia_t = mp.tile([P, 8 * secw], I16, tag="ia")
                    nc.sync.dma_start(out=ia_t[:], in_=idxa_a[:, t * 8 * secw:(t + 1) * 8 * secw])
                    ib_t = mp.tile([P, 8 * secw], I16, tag="ib")
                    nc.sync.dma_start(out=ib_t[:], in_=idxb_a[:, t * 8 * secw:(t + 1) * 8 * secw])
                    me_t = mp.tile([P, 8 * 2 * pch2], F32, tag="me")
                    nc.sync.dma_start(out=me_t[:], in_=meta_a[:, t * 16 * pch2:(t + 1) * 16 * pch2])
                    zt_sb = []
                    for r in range(R):
                        g = gp.tile([P, pch2, xcols], F32, tag="gA")
                        nc.gpsimd.dma_gather(
                            out_ap=g[:, 0:pch2 // 2, :], in_ap=xA[:, :],
                            idxs_ap=ia_t[:, r * secw:(r + 1) * secw],
                            num_idxs=sec, num_idxs_reg=sec, elem_size=xcols)
                        nc.gpsimd.dma_gather(
                            out_ap=g[:, pch2 // 2:pch2, :], in_ap=xB[:, :],
                            idxs_ap=ib_t[:, r * secw:(r + 1) * secw],
                            num_idxs=sec, num_idxs_reg=sec, elem_size=xcols)
                        z_ps0 = ps.tile([P, 512], F32, tag="zps0")
                        z_ps1 = ps.tile([P, xcols - 512], F32, tag="zps1")
                        for k in range(pch2):
                            s = sp.tile([P, P], F32, tag="smat")
                            nc.vector.tensor_scalar(
                                out=s[:], in0=iota_t[:],
                                scalar1=me_t[:, r * 2 * pch2 + k:r * 2 * pch2 + k + 1],
                                scalar2=me_t[:, r * 2 * pch2 + pch2 + k:r * 2 * pch2 + pch2 + k + 1],
                                op0=mybir.AluOpType.is_equal, op1=mybir.AluOpType.mult)
                            nc.tensor.matmul(z_ps0[:], s[:], g[:, k, 0:512],
                                             start=(k == 0), stop=(k == pch2 - 1))
                            nc.tensor.matmul(z_ps1[:], s[:], g[:, k, 512:xcols],
                                             start=(k == 0), stop=(k == pch2 - 1))
                        zs = zp.tile([P, xcols], F32, tag="zs")
                        nc.vector.tensor_copy(out=zs[:, 0:512], in_=z_ps0[:])
                        nc.vector.tensor_copy(out=zs[:, 512:xcols], in_=z_ps1[:])
                        zt_r = ztp_p.tile([P, nsl_a * P], F32, tag=f"zt{r}")
                        for k in range(nsl_a):
                            kw = ksizes_a[k]
                            ztps = ps2.tile([P, P], F32, tag="ztps")
                            nc.tensor.transpose(out=ztps[0:kw, :], in_=zs[:, k * P:k * P + kw],
                                                identity=ident[:])
                            nc.vector.tensor_copy(out=zt_r[0:kw, k * P:(k + 1) * P], in_=ztps[0:kw, :])
                        zt_sb.append(zt_r)
                    out_ps2 = ps3.tile([P, 256], F32, tag="ops2")
                    out_ps3 = ps3.tile([P, 256], F32, tag="ops3")
                    for r in range(R):
                        for k in range(nsl_a):
                            kw = ksizes_a[k]
                            tgt = out_ps2 if k < 2 else out_ps3
                            nc.tensor.matmul(
                                tgt[:], zt_sb[r][0:kw, k * P:(k + 1) * P],
                                wa_t[r * nsl_a + k][0:kw, :],
                                start=(r == 0 and k in (0, 2)),
                                stop=(r == R - 1 and k in (1, nsl_a - 1)))
                    hsb = zp.tile([P, HC], F32, tag="hsb")
                    nc.vector.tensor_tensor(out=hsb[:, 0:256], in0=out_ps2[:], in1=bias_t[:, 0:256],
                                            op=mybir.AluOpType.add)
                    nc.vector.tensor_tensor(out=hsb[:, 256:512], in0=out_ps3[:], in1=bias_t[:, 256:512],
                                            op=mybir.AluOpType.add)
                    nc.vector.tensor_scalar_max(out=hsb[:], in0=hsb[:], scalar1=0.0)
                    nc.sync.dma_start(out=h_shard[t * P:(t + 1) * P, :], in_=hsb[:])

            if DEBUG:
                nc.sync.dma_start(out=h_out[:, :], in_=h_shard[:, :])
            nc.gpsimd.collective_compute(
                "AllGather", mybir.AluOpType.bypass,
                replica_groups=[list(range(NC))],
                ins=[h_shard[:, :]], outs=[h_full[:, :]])

            # ---------------- layer B ----------------
            with (
                tc.tile_pool(name="wpoolb", bufs=1) as wp,
                tc.tile_pool(name="gpoolb", bufs=3) as gp,
                tc.tile_pool(name="spoolb", bufs=4) as sp,
                tc.tile_pool(name="zpoolb", bufs=2) as zp,
                tc.tile_pool(name="ztpoolb", bufs=2) as ztp_p,
                tc.tile_pool(name="mpoolb", bufs=2) as mp,
                tc.tile_pool(name="pspoolb", bufs=2, space="PSUM") as ps,
                tc.tile_pool(name="ps2poolb", bufs=2, space="PSUM") as ps2,
                tc.tile_pool(name="ps3poolb", bufs=2, space="PSUM") as ps3,
            ):
                wb_t = []
                for r in range(R):
                    for k in range(4):
                        w_rk = wp.tile([P, 16], F32, tag=f"wb{r}_{k}")
                        nc.sync.dma_start(out=w_rk[:], in_=WB[r, k, :, :])
                        wb_t.append(w_rk)
                for t in range(tpc):
                    ia_t = mp.tile([P, 8 * secw], I16, tag="iab")
                    nc.sync.dma_start(out=ia_t[:], in_=idxa_b[:, t * 8 * secw:(t + 1) * 8 * secw])
                    ib_t = mp.tile([P, 8 * secw], I16, tag="ibb")
                    nc.sync.dma_start(out=ib_t[:], in_=idxb_b[:, t * 8 * secw:(t + 1) * 8 * secw])
                    me_t = mp.tile([P, 8 * 2 * pch2], F32, tag="meb")
                    nc.sync.dma_start(out=me_t[:], in_=meta_b[:, t * 16 * pch2:(t + 1) * 16 * pch2])
                    uv_ps = ps3.tile([P, 16], F32, tag="uvps")
                    for r in range(R):
                        g = gp.tile([P, pch2, HC], F32, tag="gB")
                        nc.gpsimd.dma_gather(
                            out_ap=g[:, 0:pch2 // 2, :], in_ap=h_full[0:half_h, :],
                            idxs_ap=ia_t[:, r * secw:(r + 1) * secw],
                            num_idxs=sec, num_idxs_reg=sec, elem_size=HC)
                        nc.gpsimd.dma_gather(
                            out_ap=g[:, pch2 // 2:pch2, :], in_ap=h_full[half_h:npad, :],
                            idxs_ap=ib_t[:, r * secw:(r + 1) * secw],
                            num_idxs=sec, num_idxs_reg=sec, elem_size=HC)
                        z_ps = ps.tile([P, HC], F32, tag="zpsb")
                        for k in range(pch2):
                            s = sp.tile([P, P], F32, tag="smatb")
                            nc.vector.tensor_scalar(
                                out=s[:], in0=iota_t[:],
                                scalar1=me_t[:, r * 2 * pch2 + k:r * 2 * pch2 + k + 1],
                                scalar2=me_t[:, r * 2 * pch2 + pch2 + k:r * 2 * pch2 + pch2 + k + 1],
                                op0=mybir.AluOpType.is_equal, op1=mybir.AluOpType.mult)
                            nc.tensor.matmul(z_ps[:], s[:], g[:, k, :],
                                             start=(k == 0), stop=(k == pch2 - 1))
                        zs = zp.tile([P, HC], F32, tag="zsb")
                        nc.vector.tensor_copy(out=zs[:], in_=z_ps[:])
                        zt_r = ztp_p.tile([P, 4 * P], F32, tag=f"ztb{r}")
                        for k in range(4):
                            ztps = ps2.tile([P, P], F32, tag="ztpsb")
                            nc.tensor.transpose(out=ztps[:], in_=zs[:, k * P:(k + 1) * P],
                                                identity=ident[:])
                            nc.vector.tensor_copy(out=zt_r[:, k * P:(k + 1) * P], in_=ztps[:])
                        for k in range(4):
                            nc.tensor.matmul(
                                uv_ps[:], zt_r[:, k * P:(k + 1) * P], wb_t[r * 4 + k][:],
                                start=(r == 0 and k == 0), stop=(r == R - 1 and k == 3))
                    uvsb = zp.tile([P, 16], F32, tag="uvsb")
                    nc.vector.tensor_tensor(out=uvsb[:], in0=uv_ps[:], in1=uvb_t[:],
                                            op=mybir.AluOpType.add)
                    nc.sync.dma_start(out=uv_shard[t * P:(t + 1) * P, :], in_=uvsb[:])

            if DEBUG:
                nc.sync.dma_start(out=uv_out[:, :], in_=uv_shard[:, :])
            nc.gpsimd.collective_compute(
                "AllGather", mybir.AluOpType.bypass,
                replica_groups=[list(range(NC))],
                ins=[uv_shard[:, :]], outs=[uv_full[:, :]])
            # pack pairs of node rows into 256B rows for the decoder gather
            nc.sync.dma_start(
                out=uv_pack[:, 0:32],
                in_=uv_full[:, :].rearrange("(r two) c -> r (two c)", two=2))

            # ---------------- decoder ----------------
            with (
                tc.tile_pool(name="dgp", bufs=3) as gp,
                tc.tile_pool(name="dmp", bufs=2) as mp,
                tc.tile_pool(name="dvp", bufs=3) as vp,
            ):
                sview = score_out.ap().rearrange("(B j p) d -> B p j d", p=P, j=8)
                for b in range(nb):
                    iu_t = mp.tile([P, 64], I16, tag="iu")
                    nc.sync.dma_start(out=iu_t[:], in_=idx_u[:, b * 64:(b + 1) * 64])
                    iv_t = mp.tile([P, 64], I16, tag="iv")
                    nc.sync.dma_start(out=iv_t[:], in_=idx_v[:, b * 64:(b + 1) * 64])
                    mu_t = mp.tile([P, 64], F32, tag="mu")
                    nc.sync.dma_start(out=mu_t[:], in_=msk_u[:, b * 64:(b + 1) * 64])
                    mv_t = mp.tile([P, 64], F32, tag="mv")
                    nc.sync.dma_start(out=mv_t[:], in_=msk_v[:, b * 64:(b + 1) * 64])
                    gu = gp.tile([P, 8, 64], F32, tag="gu")
                    nc.gpsimd.dma_gather(out_ap=gu[:], in_ap=uv_pack[:, :], idxs_ap=iu_t[:],
                                         num_idxs=1024, num_idxs_reg=1024, elem_size=64)
                    gv = gp.tile([P, 8, 64], F32, tag="gv")
                    nc.gpsimd.dma_gather(out_ap=gv[:], in_ap=uv_pack[:, :], idxs_ap=iv_t[:],
                                         num_idxs=1024, num_idxs_reg=1024, elem_size=64)
                    muv = mu_t[:].rearrange("p (j d) -> p j d", d=8)
                    mvv = mv_t[:].rearrange("p (j d) -> p j d", d=8)
                    du = vp.tile([P, 8, 8], F32, tag="du")
                    nc.vector.tensor_tensor(out=du[:], in0=gu[:, :, 16:24], in1=gu[:, :, 0:8],
                                            op=mybir.AluOpType.subtract)
                    nc.vector.tensor_tensor(out=du[:], in0=du[:], in1=muv,
                                            op=mybir.AluOpType.mult)
                    nc.vector.tensor_tensor(out=du[:], in0=du[:], in1=gu[:, :, 0:8],
                                            op=mybir.AluOpType.add)
                    dv = vp.tile([P, 8, 8], F32, tag="dv")
                    nc.vector.tensor_tensor(out=dv[:], in0=gv[:, :, 24:32], in1=gv[:, :, 8:16],
                                            op=mybir.AluOpType.subtract)
                    nc.vector.tensor_tensor(out=dv[:], in0=dv[:], in1=mvv,
                                            op=mybir.AluOpType.mult)
                    nc.vector.tensor_tensor(out=dv[:], in0=dv[:], in1=gv[:, :, 8:16],
                                            op=mybir.AluOpType.add)
                    nc.vector.tensor_tensor(out=du[:], in0=du[:], in1=dv[:],
                                            op=mybir.AluOpType.add)
                    nc.sync.dma_start(out=sview[b], in_=du[:])
    nc.finalize()
    return nc


def _prep(inputs):
    x2 = np.asarray(inputs["node2_features"], np.float32)
    x3 = np.asarray(inputs["mpnn_features"], np.float32)
    src = np.asarray(inputs["src"])
    dst = np.asarray(inputs["dst"])
    dec_src = np.asarray(inputs["dec_src"]).astype(np.int64)
    dec_dst = np.asarray(inputs["dec_dst"]).astype(np.int64)
    W2a = np.asarray(inputs["W2a"], np.float32)
    b2a = np.asarray(inputs["b2a"], np.float32)
    W2b = np.asarray(inputs["W2b"], np.float32)
    b2b = np.asarray(inputs["b2b"], np.float32)
    W3a = np.asarray(inputs["W3a"], np.float32)
    b3a = np.asarray(inputs["b3a"], np.float32)
    W3b = np.asarray(inputs["W3b"], np.float32)
    b3b = np.asarray(inputs["b3b"], np.float32)
    Wp1 = np.asarray(inputs["Wp1"], np.float32)
    bp1 = np.asarray(inputs["bp1"], np.float32)
    Wp2 = np.asarray(inputs["Wp2"], np.float32)
    bp2 = np.asarray(inputs["bp2"], np.float32)

    n = x2.shape[0]
    R, E = src.shape
    assert R == 8
    ed = dec_src.shape[0]
    d2, d3 = x2.shape[1], x3.shape[1]
    assert d2 == 256 and W2a.shape == (8, 256, 256)
    dcat = d2 + d3
    xcols = -(-dcat * 4 // 256) * 64          # pad row to multiple of 256B
    nsl_a = -(-xcols // P)
    ksizes_a = [min(P, xcols - k * P) for k in range(nsl_a)]
    ntiles = NC * (-(-n // (P * NC)))
    tpc = ntiles // NC
    npad = ntiles * P
    split_x = (n + 1) // 2
    assert max(split_x, n - split_x, npad // 2) <= 32767

    # per-edge weights
    ns_arr = np.stack([_deg_norm(src[r], n) for r in range(R)])
    nd_arr = np.stack([_deg_norm(dst[r], n) for r in range(R)])
    src_f = src.astype(np.int64).ravel()
    dst_f = dst.astype(np.int64).ravel()
    rel_f = np.repeat(np.arange(R, dtype=np.int64), E)
    w_f = (ns_arr[rel_f, src_f] * nd_arr[rel_f, dst_f]).astype(np.float32)

    half_h = npad // 2
    counts_a = np.bincount((dst_f >> 7) * 16 + rel_f * 2 + (src_f >= split_x),
                           minlength=ntiles * 16)
    counts_b = np.bincount((dst_f >> 7) * 16 + rel_f * 2 + (src_f >= half_h),
                           minlength=ntiles * 16)
    sec = P * max(1, -(-int(max(counts_a.max(), counts_b.max())) // P))

    idxa_a, idxb_a, meta_a = _pack_edges(src_f, rel_f, dst_f, w_f, split_x, tpc, sec, ntiles)
    idxa_b, idxb_b, meta_b = _pack_edges(src_f, rel_f, dst_f, w_f, half_h, tpc, sec, ntiles)

    # tables
    x_cat = np.zeros((n, xcols), np.float32)
    x_cat[:, :d2] = x2
    x_cat[:, d2:dcat] = x3
    xA = np.ascontiguousarray(x_cat[:split_x])
    xB = np.ascontiguousarray(x_cat[split_x:])

    # layer-A weight slices [R, nsl_a, 128, 256]
    WAp = np.zeros((R, nsl_a, P, 256), np.float32)
    for r in range(R):
        for k in range(nsl_a):
            lo = k * P
            for rr in range(ksizes_a[k]):
                f = lo + rr
                if f < d2:
                    WAp[r, k, rr, 0:256] = W2a[r, f]
                elif f < dcat:
                    WAp[r, k, rr, 0:256] = W3a[r, f - d2]

    # decoder folding
    M = Wp1 @ Wp2                     # [512, 8]
    A2, A3, B2, B3 = M[0:128], M[128:256], M[256:384], M[384:512]
    WBp = np.zeros((R, 4, P, 16), np.float32)
    for r in range(R):
        W2r = W2b[r] @ np.concatenate([A2, B2], axis=1)   # [256, 16]
        W3r = W3b[r] @ np.concatenate([A3, B3], axis=1)
        WBp[r, 0] = W2r[0:128]
        WBp[r, 1] = W2r[128:256]
        WBp[r, 2] = W3r[0:128]
        WBp[r, 3] = W3r[128:256]
    c_total = (b2b.sum(0) @ np.concatenate([A2, B2], axis=1)
               + b3b.sum(0) @ np.concatenate([A3, B3], axis=1))
    c_total = c_total[0:8] + c_total[8:16] + bp1 @ Wp2 + bp2
    uvb_rep = np.tile(np.concatenate([np.zeros(8, np.float32),
                                      c_total.astype(np.float32)]), (P, 1))
    bias_rep = np.tile(np.concatenate([b2a.sum(0), b3a.sum(0)]).astype(np.float32), (P, 1))
    iota_rep = np.tile(np.arange(P, dtype=np.float32), (P, 1))

    # decoder edges
    epc = -(-ed // NC)
    nb = -(-epc // 1024)
    in_maps = []
    for c in range(NC):
        e0 = c * epc
        s_pad = np.zeros(nb * 1024, np.int64)
        d_pad = np.zeros(nb * 1024, np.int64)
        seg = slice(e0, min(e0 + epc, ed))
        ln = seg.stop - seg.start
        s_pad[:ln] = dec_src[seg]
        d_pad[:ln] = dec_dst[seg]
        iu = _wrap16((s_pad >> 1).astype(np.int16).reshape(nb, 1024))
        iv = _wrap16((d_pad >> 1).astype(np.int16).reshape(nb, 1024))
        mu = (s_pad & 1).astype(np.float32).reshape(nb, 8, P).transpose(2, 0, 1)
        mv = (d_pad & 1).astype(np.float32).reshape(nb, 8, P).transpose(2, 0, 1)
        mu = np.repeat(mu.reshape(P, nb * 8), 8, axis=1)
        mv = np.repeat(mv.reshape(P, nb * 8), 8, axis=1)
        in_maps.append(dict(
            xA=xA, xB=xB, WA=WAp, WB=WBp, bias_rep=bias_rep, uvb_rep=uvb_rep,
            iota_rep=iota_rep,
            idxa_a=idxa_a[c], idxb_a=idxb_a[c], meta_a=meta_a[c],
            idxa_b=idxa_b[c], idxb_b=idxb_b[c], meta_b=meta_b[c],
            idx_u=np.ascontiguousarray(np.tile(iu, (8, 1))),
            idx_v=np.ascontiguousarray(np.tile(iv, (8, 1))),
            msk_u=np.ascontiguousarray(mu), msk_v=np.ascontiguousarray(mv),
        ))
    dims = dict(n=n, tpc=tpc, sec=sec, xcols=xcols, nsl_a=nsl_a,
                ksizes_a=ksizes_a, epc=epc, nb=nb, split_x=split_x, ed=ed)
    return in_maps, dims


_CACHE = {}


def kernel(**inputs):
    in_maps, dims = _prep(inputs)
    key = (dims["n"], dims["tpc"], dims["sec"], dims["xcols"], dims["nb"])
    nc = _CACHE.get(key)
    if nc is None:
        nc = _build(dims)
        _CACHE[key] = nc
    res = run_bass_kernel_spmd(nc, in_maps, list(range(NC)))
    epc, ed = dims["epc"], dims["ed"]
    out = np.concatenate(
        [res.results[c]["score_out"][:min(epc, ed - c * epc)] for c in range(NC)], axis=0)
    return np.ascontiguousarray(out.astype(np.float32))


if __name__ == "__main__":
    pass


# revision 4
# speedup vs baseline: 1.0289x; 1.0289x over previous
"""Trainium2 Bass kernel for the 2-layer hetero-GCN + linear edge decoder.

Math restructuring (exact, up to fp reassociation):
  hetero_conv: out = sum_r nd_r*(A_r @ (ns_r*x)) @ W_r + sum_r b_r
    -> per-edge weight w_e = ns_r[src]*nd_r[dst] folded into a one-hot
       scatter matrix S; aggregation z_r = S^T @ gathered(x) runs on the
       TensorEngine; the W_r matmul happens once per dst-tile.
  decoder has NO nonlinearity between Wp1 and Wp2, so
    score[e] = u[src_e] + v[dst_e] + c,  u = feat @ (Wp1@Wp2)[:256],
    v = feat @ (Wp1@Wp2)[256:], folded into the layer-B weight matmul.

Sharding: dst-nodes partitioned into 128-node tiles, 49 tiles/core x 8
cores; x/h tables replicated; AllGather of the layer-A activations and of
the tiny (u,v) table; decoder edges sharded evenly.
"""
import os
import sys

for _p in ("/opt/trn_rl_repo", "/root/.axon_site/_ro/trn_rl_repo"):
    if os.path.isdir(_p) and _p not in sys.path:
        sys.path.append(_p)

import numpy as np

import concourse.bass as bass
import concourse.bacc as bacc
import concourse.mybir as mybir
import concourse.tile as tile
from concourse.bass_utils import run_bass_kernel_spmd
from concourse.masks import make_identity

P = 128
NC = 8
F32 = mybir.dt.float32
DEBUG = False
I16 = mybir.dt.int16


def _deg_norm(idx, n):
    deg = np.bincount(idx, minlength=n).astype(np.float32)
    out = np.zeros(n, np.float32)
    nz = deg > 0
    out[nz] = 1.0 / np.sqrt(np.maximum(deg[nz], 1.0))
    return out


def _wrap16(a):
    # [G, S] int16 -> [16, G*S/16] wrapped (idx i of each group -> [i%16, i//16])
    G, S = a.shape
    return a.reshape(G, S // 16, 16).transpose(2, 0, 1).reshape(16, G * (S // 16))


def _pack_edges(src_f, rel_f, dst_f, w_f, split, tpc, sec, ntiles):
    """Pack edges into per-core gather-idx + meta arrays.

    Returns idxA, idxB [NC][128, tpc*8*sec/16] int16 (wrapped+replicated),
    meta [NC][128, tpc*8*2*pch] f32 (dst_local chunks | w chunks).
    """
    pch2 = 2 * sec // P
    tile_id = dst_f >> 7
    half = (src_f >= split).astype(np.int64)
    order = np.lexsort((half, rel_f, tile_id))
    so, ro, do_, wo, ho = (a[order] for a in (src_f, rel_f, dst_f, w_f, half))
    to = tile_id[order]
    key = (to * 8 + ro) * 2 + ho
    ngroup = ntiles * 8 * 2
    counts = np.bincount(key, minlength=ngroup)
    assert counts.max() <= sec, (counts.max(), sec)
    starts = np.zeros(ngroup, np.int64)
    np.cumsum(counts[:-1], out=starts[1:])
    rank = np.arange(len(so)) - starts[key]
    # slot within (tile, rel) group of 2*sec
    gslot = ho * sec + rank
    gidx = to * 8 + ro  # global group id (tile-major)
    idx_val = (so - ho * split).astype(np.int16)

    idxA = np.zeros((ntiles * 8, sec), np.int16)
    idxB = np.zeros((ntiles * 8, sec), np.int16)
    mA = ho == 0
    idxA[gidx[mA], rank[mA]] = idx_val[mA]
    idxB[gidx[~mA], rank[~mA]] = idx_val[~mA]
    dst_local = np.zeros((ntiles * 8, 2 * sec), np.float32)
    w_slot = np.zeros((ntiles * 8, 2 * sec), np.float32)
    dst_local[gidx, gslot] = (do_ & 127).astype(np.float32)
    w_slot[gidx, gslot] = wo

    gpc = tpc * 8  # groups per core
    idxA_c, idxB_c, meta_c = [], [], []
    for c in range(NC):
        sl = slice(c * gpc, (c + 1) * gpc)
        ia = np.tile(_wrap16(idxA[sl]), (8, 1))
        ib = np.tile(_wrap16(idxB[sl]), (8, 1))
        # meta: per group block of 2*pch2 cols: [dst chunks pch2 | w chunks pch2]
        d = dst_local[sl].reshape(gpc, pch2, P).transpose(2, 0, 1)  # [128, gpc, pch2]
        w_ = w_slot[sl].reshape(gpc, pch2, P).transpose(2, 0, 1)
        meta = np.concatenate([d, w_], axis=2).reshape(P, gpc * 2 * pch2)
        idxA_c.append(np.ascontiguousarray(ia))
        idxB_c.append(np.ascontiguousarray(ib))
        meta_c.append(np.ascontiguousarray(meta))
    return idxA_c, idxB_c, meta_c


def _build(dims):
    (tpc, sec, xcols, nsl_a, ksizes_a, epc, nb) = (
        dims["tpc"], dims["sec"], dims["xcols"], dims["nsl_a"],
        dims["ksizes_a"], dims["epc"], dims["nb"])
    ntiles = tpc * NC
    npad = ntiles * P
    half_h = npad // 2
    split_x = dims["split_x"]
    pch2 = 2 * sec // P
    secw = sec // 16
    HC = 512
    R = 8
    nc = bacc.Bacc("TRN2", target_bir_lowering=False, debug=False)

    xA = nc.declare_dram_parameter("xA", [split_x, xcols], F32, isOutput=False)
    xB = nc.declare_dram_parameter("xB", [dims["n"] - split_x, xcols], F32, isOutput=False)
    WA = nc.declare_dram_parameter("WA", [R, nsl_a, P, 256], F32, isOutput=False)
    WM = nc.declare_dram_parameter("WM", [4, P, P], F32, isOutput=False)
    bias_rep = nc.declare_dram_parameter("bias_rep", [P, HC], F32, isOutput=False)
    uvb_rep = nc.declare_dram_parameter("uvb_rep", [P, 16], F32, isOutput=False)
    iota_rep = nc.declare_dram_parameter("iota_rep", [P, P], F32, isOutput=False)
    idxa_a = nc.declare_dram_parameter("idxa_a", [P, tpc * 8 * secw], I16, isOutput=False)
    idxb_a = nc.declare_dram_parameter("idxb_a", [P, tpc * 8 * secw], I16, isOutput=False)
    meta_a = nc.declare_dram_parameter("meta_a", [P, tpc * 8 * 2 * pch2], F32, isOutput=False)
    idxa_b = nc.declare_dram_parameter("idxa_b", [P, tpc * 8 * secw], I16, isOutput=False)
    idxb_b = nc.declare_dram_parameter("idxb_b", [P, tpc * 8 * secw], I16, isOutput=False)
    meta_b = nc.declare_dram_parameter("meta_b", [P, tpc * 8 * 2 * pch2], F32, isOutput=False)
    idx_u = nc.declare_dram_parameter("idx_u", [P, nb * 64], I16, isOutput=False)
    idx_v = nc.declare_dram_parameter("idx_v", [P, nb * 64], I16, isOutput=False)
    msk_u = nc.declare_dram_parameter("msk_u", [P, nb * 64], F32, isOutput=False)
    msk_v = nc.declare_dram_parameter("msk_v", [P, nb * 64], F32, isOutput=False)
    score_out = nc.declare_dram_parameter("score_out", [nb * 1024, 8], F32, isOutput=True)
    if DEBUG:
        h_out = nc.declare_dram_parameter("h_out", [tpc * P, 512], F32, isOutput=True)
        uv_out = nc.declare_dram_parameter("uv_out", [tpc * P, 16], F32, isOutput=True)

    with tile.TileContext(nc) as tc:
        with (
            tc.tile_pool(name="cpool", bufs=1) as cp,
            tc.tile_pool(name="dram", bufs=1, space="DRAM") as dp,
        ):
            m_shard = dp.tile([tpc * P, P], F32)
            m_full = dp.tile([npad, P], F32, addr_space="Shared")
            uv_shard = dp.tile([tpc * P, 16], F32)
            uv_full = dp.tile([npad, 16], F32, addr_space="Shared")
            uv_pack = dp.tile([npad // 2, 64], F32)

            ident = cp.tile([P, P], F32)
            make_identity(nc, ident[:])
            iota_t = cp.tile([P, P], F32)
            nc.sync.dma_start(out=iota_t[:], in_=iota_rep[:, :])
            bias_t = cp.tile([P, HC], F32)
            nc.sync.dma_start(out=bias_t[:], in_=bias_rep[:, :])
            uvb_t = cp.tile([P, 16], F32)
            nc.sync.dma_start(out=uvb_t[:], in_=uvb_rep[:, :])

            # ---------------- layer A ----------------
            with (
                tc.tile_pool(name="wpool", bufs=1) as wp,
                tc.tile_pool(name="gpool", bufs=3) as gp,
                tc.tile_pool(name="spool", bufs=4) as sp,
                tc.tile_pool(name="zpool", bufs=2) as zp,
                tc.tile_pool(name="ztpool", bufs=2) as ztp_p,
                tc.tile_pool(name="mpool", bufs=2) as mp,
                tc.tile_pool(name="pspool", bufs=2, space="PSUM") as ps,
                tc.tile_pool(name="ps2pool", bufs=2, space="PSUM") as ps2,
                tc.tile_pool(name="ps3pool", bufs=1, space="PSUM") as ps3,
            ):
                wa_t = []
                for r in range(R):
                    for k in range(nsl_a):
                        w_rk = wp.tile([P, 256], F32, tag=f"wa{r}_{k}")
                        nc.sync.dma_start(out=w_rk[:], in_=WA[r, k, :, :])
                        wa_t.append(w_rk)
                wm_t = []
                for k in range(4):
                    wm_k = wp.tile([P, P], F32, tag=f"wm{k}")
                    nc.sync.dma_start(out=wm_k[:], in_=WM[k, :, :])
                    wm_t.append(wm_k)
                for t in range(tpc):
                    ia_t = mp.tile([P, 8 * secw], I16, tag="ia")
                    nc.sync.dma_start(out=ia_t[:], in_=idxa_a[:, t * 8 * secw:(t + 1) * 8 * secw])
                    ib_t = mp.tile([P, 8 * secw], I16, tag="ib")
                    nc.sync.dma_start(out=ib_t[:], in_=idxb_a[:, t * 8 * secw:(t + 1) * 8 * secw])
                    me_t = mp.tile([P, 8 * 2 * pch2], F32, tag="me")
                    nc.sync.dma_start(out=me_t[:], in_=meta_a[:, t * 16 * pch2:(t + 1) * 16 * pch2])
                    zt_sb = []
                    for r in range(R):
                        g = gp.tile([P, pch2, xcols], F32, tag="gA")
                        nc.gpsimd.dma_gather(
                            out_ap=g[:, 0:pch2 // 2, :], in_ap=xA[:, :],
                            idxs_ap=ia_t[:, r * secw:(r + 1) * secw],
                            num_idxs=sec, num_idxs_reg=sec, elem_size=xcols)
                        nc.gpsimd.dma_gather(
                            out_ap=g[:, pch2 // 2:pch2, :], in_ap=xB[:, :],
                            idxs_ap=ib_t[:, r * secw:(r + 1) * secw],
                            num_idxs=sec, num_idxs_reg=sec, elem_size=xcols)
                        z_ps0 = ps.tile([P, 512], F32, tag="zps0")
                        z_ps1 = ps.tile([P, xcols - 512], F32, tag="zps1")
                        for k in range(pch2):
                            s = sp.tile([P, P], F32, tag="smat")
                            nc.vector.tensor_scalar(
                                out=s[:], in0=iota_t[:],
                                scalar1=me_t[:, r * 2 * pch2 + k:r * 2 * pch2 + k + 1],
                                scalar2=me_t[:, r * 2 * pch2 + pch2 + k:r * 2 * pch2 + pch2 + k + 1],
                                op0=mybir.AluOpType.is_equal, op1=mybir.AluOpType.mult)
                            nc.tensor.matmul(z_ps0[:], s[:], g[:, k, 0:512],
                                             start=(k == 0), stop=(k == pch2 - 1))
                            nc.tensor.matmul(z_ps1[:], s[:], g[:, k, 512:xcols],
                                             start=(k == 0), stop=(k == pch2 - 1))
                        zs = zp.tile([P, xcols], F32, tag="zs")
                        nc.vector.tensor_copy(out=zs[:, 0:512], in_=z_ps0[:])
                        nc.vector.tensor_copy(out=zs[:, 512:xcols], in_=z_ps1[:])
                        zt_r = ztp_p.tile([P, nsl_a * P], F32, tag=f"zt{r}")
                        for k in range(nsl_a):
                            kw = ksizes_a[k]
                            ztps = ps2.tile([P, P], F32, tag="ztps")
                            nc.tensor.transpose(out=ztps[0:kw, :], in_=zs[:, k * P:k * P + kw],
                                                identity=ident[:])
                            nc.vector.tensor_copy(out=zt_r[0:kw, k * P:(k + 1) * P], in_=ztps[0:kw, :])
                        zt_sb.append(zt_r)
                    out_ps2 = ps3.tile([P, 256], F32, tag="ops2")
                    out_ps3 = ps3.tile([P, 256], F32, tag="ops3")
                    for r in range(R):
                        for k in range(nsl_a):
                            kw = ksizes_a[k]
                            tgt = out_ps2 if k < 2 else out_ps3
                            nc.tensor.matmul(
                                tgt[:], zt_sb[r][0:kw, k * P:(k + 1) * P],
                                wa_t[r * nsl_a + k][0:kw, :],
                                start=(r == 0 and k in (0, 2)),
                                stop=(r == R - 1 and k in (1, nsl_a - 1)))
                    hsb = zp.tile([P, HC], F32, tag="hsb")
                    nc.vector.tensor_tensor(out=hsb[:, 0:256], in0=out_ps2[:], in1=bias_t[:, 0:256],
                                            op=mybir.AluOpType.add)
                    nc.vector.tensor_tensor(out=hsb[:, 256:512], in0=out_ps3[:], in1=bias_t[:, 256:512],
                                            op=mybir.AluOpType.add)
                    nc.vector.tensor_scalar_max(out=hsb[:], in0=hsb[:], scalar1=0.0)
                    if DEBUG:
                        nc.sync.dma_start(out=h_out[t * P:(t + 1) * P, :], in_=hsb[:])
                    # m = h @ WMcat  (fold layer-B weights + decoder projection)
                    m_ps = ps3.tile([P, P], F32, tag="ops2")
                    for k in range(4):
                        htps = ps2.tile([P, P], F32, tag="ztps")
                        nc.tensor.transpose(out=htps[:], in_=hsb[:, k * P:(k + 1) * P],
                                            identity=ident[:])
                        hts = zp.tile([P, P], F32, tag="hts")
                        nc.vector.tensor_copy(out=hts[:], in_=htps[:])
                        nc.tensor.matmul(m_ps[:], hts[:], wm_t[k][:],
                                         start=(k == 0), stop=(k == 3))
                    msb = zp.tile([P, P], F32, tag="msb")
                    nc.vector.tensor_copy(out=msb[:], in_=m_ps[:])
                    nc.sync.dma_start(out=m_shard[t * P:(t + 1) * P, :], in_=msb[:])

            nc.gpsimd.collective_compute(
                "AllGather", mybir.AluOpType.bypass,
                replica_groups=[list(range(NC))],
                ins=[m_shard[:, :]], outs=[m_full[:, :]])

            # ---------------- layer B (m-space aggregation) ----------------
            with (
                tc.tile_pool(name="gpoolb", bufs=3) as gp,
                tc.tile_pool(name="spoolb", bufs=4) as sp,
                tc.tile_pool(name="zpoolb", bufs=2) as zp,
                tc.tile_pool(name="mpoolb", bufs=2) as mp,
                tc.tile_pool(name="ps3poolb", bufs=2, space="PSUM") as ps3,
            ):
                for t in range(tpc):
                    ia_t = mp.tile([P, 8 * secw], I16, tag="iab")
                    nc.sync.dma_start(out=ia_t[:], in_=idxa_b[:, t * 8 * secw:(t + 1) * 8 * secw])
                    ib_t = mp.tile([P, 8 * secw], I16, tag="ibb")
                    nc.sync.dma_start(out=ib_t[:], in_=idxb_b[:, t * 8 * secw:(t + 1) * 8 * secw])
                    me_t = mp.tile([P, 8 * 2 * pch2], F32, tag="meb")
                    nc.sync.dma_start(out=me_t[:], in_=meta_b[:, t * 16 * pch2:(t + 1) * 16 * pch2])
                    uv_ps = ps3.tile([P, 16], F32, tag="uvps")
                    for r in range(R):
                        g = gp.tile([P, pch2, P], F32, tag="gB")
                        nc.gpsimd.dma_gather(
                            out_ap=g[:, 0:pch2 // 2, :], in_ap=m_full[0:half_h, :],
                            idxs_ap=ia_t[:, r * secw:(r + 1) * secw],
                            num_idxs=sec, num_idxs_reg=sec, elem_size=P)
                        nc.gpsimd.dma_gather(
                            out_ap=g[:, pch2 // 2:pch2, :], in_ap=m_full[half_h:npad, :],
                            idxs_ap=ib_t[:, r * secw:(r + 1) * secw],
                            num_idxs=sec, num_idxs_reg=sec, elem_size=P)
                        for k in range(pch2):
                            s = sp.tile([P, P], F32, tag="smatb")
                            nc.vector.tensor_scalar(
                                out=s[:], in0=iota_t[:],
                                scalar1=me_t[:, r * 2 * pch2 + k:r * 2 * pch2 + k + 1],
                                scalar2=me_t[:, r * 2 * pch2 + pch2 + k:r * 2 * pch2 + pch2 + k + 1],
                                op0=mybir.AluOpType.is_equal, op1=mybir.AluOpType.mult)
                            nc.tensor.matmul(uv_ps[:], s[:], g[:, k, r * 16:(r + 1) * 16],
                                             start=(r == 0 and k == 0),
                                             stop=(r == R - 1 and k == pch2 - 1))
                    uvsb = zp.tile([P, 16], F32, tag="uvsb")
                    nc.vector.tensor_tensor(out=uvsb[:], in0=uv_ps[:], in1=uvb_t[:],
                                            op=mybir.AluOpType.add)
                    nc.sync.dma_start(out=uv_shard[t * P:(t + 1) * P, :], in_=uvsb[:])

            nc.gpsimd.collective_compute(
                "AllGather", mybir.AluOpType.bypass,
                replica_groups=[list(range(NC))],
                ins=[uv_shard[:, :]], outs=[uv_full[:, :]])
            # pack pairs of node rows into 256B rows for the decoder gather
            nc.sync.dma_start(
                out=uv_pack[:, 0:32],
                in_=uv_full[:, :].rearrange("(r two) c -> r (two c)", two=2))

            # ---------------- decoder ----------------
            with (
                tc.tile_pool(name="dgp", bufs=3) as gp,
                tc.tile_pool(name="dmp", bufs=2) as mp,
                tc.tile_pool(name="dvp", bufs=3) as vp,
            ):
                sview = score_out.ap().rearrange("(B j p) d -> B p j d", p=P, j=8)
                for b in range(nb):
                    iu_t = mp.tile([P, 64], I16, tag="iu")
                    nc.sync.dma_start(out=iu_t[:], in_=idx_u[:, b * 64:(b + 1) * 64])
                    iv_t = mp.tile([P, 64], I16, tag="iv")
                    nc.sync.dma_start(out=iv_t[:], in_=idx_v[:, b * 64:(b + 1) * 64])
                    mu_t = mp.tile([P, 64], F32, tag="mu")
                    nc.sync.dma_start(out=mu_t[:], in_=msk_u[:, b * 64:(b + 1) * 64])
                    mv_t = mp.tile([P, 64], F32, tag="mv")
                    nc.sync.dma_start(out=mv_t[:], in_=msk_v[:, b * 64:(b + 1) * 64])
                    gu = gp.tile([P, 8, 64], F32, tag="gu")
                    nc.gpsimd.dma_gather(out_ap=gu[:], in_ap=uv_pack[:, :], idxs_ap=iu_t[:],
                                         num_idxs=1024, num_idxs_reg=1024, elem_size=64)
                    gv = gp.tile([P, 8, 64], F32, tag="gv")
                    nc.gpsimd.dma_gather(out_ap=gv[:], in_ap=uv_pack[:, :], idxs_ap=iv_t[:],
                                         num_idxs=1024, num_idxs_reg=1024, elem_size=64)
                    muv = mu_t[:].rearrange("p (j d) -> p j d", d=8)
                    mvv = mv_t[:].rearrange("p (j d) -> p j d", d=8)
                    du = vp.tile([P, 8, 8], F32, tag="du")
                    nc.vector.tensor_tensor(out=du[:], in0=gu[:, :, 16:24], in1=gu[:, :, 0:8],
                                            op=mybir.AluOpType.subtract)
                    nc.vector.tensor_tensor(out=du[:], in0=du[:], in1=muv,
                                            op=mybir.AluOpType.mult)
                    nc.vector.tensor_tensor(out=du[:], in0=du[:], in1=gu[:, :, 0:8],
                                            op=mybir.AluOpType.add)
                    dv = vp.tile([P, 8, 8], F32, tag="dv")
                    nc.vector.tensor_tensor(out=dv[:], in0=gv[:, :, 24:32], in1=gv[:, :, 8:16],
                                            op=mybir.AluOpType.subtract)
                    nc.vector.tensor_tensor(out=dv[:], in0=dv[:], in1=mvv,
                                            op=mybir.AluOpType.mult)
                    nc.vector.tensor_tensor(out=dv[:], in0=dv[:], in1=gv[:, :, 8:16],
                                            op=mybir.AluOpType.add)
                    nc.vector.tensor_tensor(out=du[:], in0=du[:], in1=dv[:],
                                            op=mybir.AluOpType.add)
                    nc.sync.dma_start(out=sview[b], in_=du[:])
    nc.finalize()
    return nc


def _prep(inputs):
    x2 = np.asarray(inputs["node2_features"], np.float32)
    x3 = np.asarray(inputs["mpnn_features"], np.float32)
    src = np.asarray(inputs["src"])
    dst = np.asarray(inputs["dst"])
    dec_src = np.asarray(inputs["dec_src"]).astype(np.int64)
    dec_dst = np.asarray(inputs["dec_dst"]).astype(np.int64)
    W2a = np.asarray(inputs["W2a"], np.float32)
    b2a = np.asarray(inputs["b2a"], np.float32)
    W2b = np.asarray(inputs["W2b"], np.float32)
    b2b = np.asarray(inputs["b2b"], np.float32)
    W3a = np.asarray(inputs["W3a"], np.float32)
    b3a = np.asarray(inputs["b3a"], np.float32)
    W3b = np.asarray(inputs["W3b"], np.float32)
    b3b = np.asarray(inputs["b3b"], np.float32)
    Wp1 = np.asarray(inputs["Wp1"], np.float32)
    bp1 = np.asarray(inputs["bp1"], np.float32)
    Wp2 = np.asarray(inputs["Wp2"], np.float32)
    bp2 = np.asarray(inputs["bp2"], np.float32)

    n = x2.shape[0]
    R, E = src.shape
    assert R == 8
    ed = dec_src.shape[0]
    d2, d3 = x2.shape[1], x3.shape[1]
    assert d2 == 256 and W2a.shape == (8, 256, 256)
    dcat = d2 + d3
    xcols = -(-dcat * 4 // 256) * 64          # pad row to multiple of 256B
    nsl_a = -(-xcols // P)
    ksizes_a = [min(P, xcols - k * P) for k in range(nsl_a)]
    ntiles = NC * (-(-n // (P * NC)))
    tpc = ntiles // NC
    npad = ntiles * P
    split_x = (n + 1) // 2
    assert max(split_x, n - split_x, npad // 2) <= 32767

    # per-edge weights
    ns_arr = np.stack([_deg_norm(src[r], n) for r in range(R)])
    nd_arr = np.stack([_deg_norm(dst[r], n) for r in range(R)])
    src_f = src.astype(np.int64).ravel()
    dst_f = dst.astype(np.int64).ravel()
    rel_f = np.repeat(np.arange(R, dtype=np.int64), E)
    w_f = (ns_arr[rel_f, src_f] * nd_arr[rel_f, dst_f]).astype(np.float32)

    half_h = npad // 2
    counts_a = np.bincount((dst_f >> 7) * 16 + rel_f * 2 + (src_f >= split_x),
                           minlength=ntiles * 16)
    counts_b = np.bincount((dst_f >> 7) * 16 + rel_f * 2 + (src_f >= half_h),
                           minlength=ntiles * 16)
    sec = P * max(1, -(-int(max(counts_a.max(), counts_b.max())) // P))

    idxa_a, idxb_a, meta_a = _pack_edges(src_f, rel_f, dst_f, w_f, split_x, tpc, sec, ntiles)
    idxa_b, idxb_b, meta_b = _pack_edges(src_f, rel_f, dst_f, w_f, half_h, tpc, sec, ntiles)

    # tables
    x_cat = np.zeros((n, xcols), np.float32)
    x_cat[:, :d2] = x2
    x_cat[:, d2:dcat] = x3
    xA = np.ascontiguousarray(x_cat[:split_x])
    xB = np.ascontiguousarray(x_cat[split_x:])

    # layer-A weight slices [R, nsl_a, 128, 256]
    WAp = np.zeros((R, nsl_a, P, 256), np.float32)
    for r in range(R):
        for k in range(nsl_a):
            lo = k * P
            for rr in range(ksizes_a[k]):
                f = lo + rr
                if f < d2:
                    WAp[r, k, rr, 0:256] = W2a[r, f]
                elif f < dcat:
                    WAp[r, k, rr, 0:256] = W3a[r, f - d2]

    # decoder folding
    M = Wp1 @ Wp2                     # [512, 8]
    A2, A3, B2, B3 = M[0:128], M[128:256], M[256:384], M[384:512]
    WMcat = np.zeros((512, P), np.float32)
    for r in range(R):
        W2r = W2b[r] @ np.concatenate([A2, B2], axis=1)   # [256, 16]
        W3r = W3b[r] @ np.concatenate([A3, B3], axis=1)
        WMcat[0:256, r * 16:(r + 1) * 16] = W2r
        WMcat[256:512, r * 16:(r + 1) * 16] = W3r
    WMp = WMcat.reshape(4, P, P)
    c_total = (b2b.sum(0) @ np.concatenate([A2, B2], axis=1)
               + b3b.sum(0) @ np.concatenate([A3, B3], axis=1))
    c_total = c_total[0:8] + c_total[8:16] + bp1 @ Wp2 + bp2
    uvb_rep = np.tile(np.concatenate([np.zeros(8, np.float32),
                                      c_total.astype(np.float32)]), (P, 1))
    bias_rep = np.tile(np.concatenate([b2a.sum(0), b3a.sum(0)]).astype(np.float32), (P, 1))
    iota_rep = np.tile(np.arange(P, dtype=np.float32), (P, 1))

    # decoder edges
    epc = -(-ed // NC)
    nb = -(-epc // 1024)
    in_maps = []
    for c in range(NC):
        e0 = c * epc
        s_pad = np.zeros(nb * 1024, np.int64)
        d_pad = np.zeros(nb * 1024, np.int64)
        seg = slice(e0, min(e0 + epc, ed))
        ln = seg.stop - seg.start
        s_pad[:ln] = dec_src[seg]
        d_pad[:ln] = dec_dst[seg]
        iu = _wrap16((s_pad >> 1).astype(np.int16).reshape(nb, 1024))
        iv = _wrap16((d_pad >> 1).astype(np.int16).reshape(nb, 1024))
        mu = (s_pad & 1).astype(np.float32).reshape(nb, 8, P).transpose(2, 0, 1)
        mv = (d_pad & 1).astype(np.float32).reshape(nb, 8, P).transpose(2, 0, 1)
        mu = np.repeat(mu.reshape(P, nb * 8), 8, axis=1)
        mv = np.repeat(mv.reshape(P, nb * 8), 8, axis=1)
        in_maps.append(dict(
            xA=xA, xB=xB, WA=WAp, WM=WMp, bias_rep=bias_rep, uvb_rep=uvb_rep,
            iota_rep=iota_rep,
            idxa_a=idxa_a[c], idxb_a=idxb_a[c], meta_a=meta_a[c],
            idxa_b=idxa_b[c], idxb_b=idxb_b[c], meta_b=meta_b[c],
            idx_u=np.ascontiguousarray(np.tile(iu, (8, 1))),
            idx_v=np.ascontiguousarray(np.tile(iv, (8, 1))),
            msk_u=np.ascontiguousarray(mu), msk_v=np.ascontiguousarray(mv),
        ))
    dims = dict(n=n, tpc=tpc, sec=sec, xcols=xcols, nsl_a=nsl_a,
                ksizes_a=ksizes_a, epc=epc, nb=nb, split_x=split_x, ed=ed)
    return in_maps, dims


_CACHE = {}


def kernel(**inputs):
    in_maps, dims = _prep(inputs)
    key = (dims["n"], dims["tpc"], dims["sec"], dims["xcols"], dims["nb"])
    nc = _CACHE.get(key)
    if nc is None:
        nc = _build(dims)
        _CACHE[key] = nc
    res = run_bass_kernel_spmd(nc, in_maps, list(range(NC)))
    epc, ed = dims["epc"], dims["ed"]
    out = np.concatenate(
        [res.results[c]["score_out"][:min(epc, ed - c * epc)] for c in range(NC)], axis=0)
    return np.ascontiguousarray(out.astype(np.float32))


if __name__ == "__main__":
    pass
